# revision 15
# baseline (speedup 1.0000x reference)
"""EdgeNetwork GNN message-passing kernel for 8 Trainium2 NeuronCores.

Math (per batch b):
    bo = Ro[b]^T @ X[b]            # [E, F]  gather of outgoing-node feats
    bi = Ri[b]^T @ X[b]            # [E, F]
    feats = concat(bo, bi)         # [E, 2F]
    h = tanh(feats @ W1 + b1)      # [E, H]
    out = sigmoid(h @ W2 + b2)     # [E]

Sharding: 8 shards over (B=2) x (E/4): each core owns one (b, 2048-edge)
slice with zero cross-core communication.  Per core we stream its
Ro/Ri slices ([4096, 2048] f32, 16 MB each) chunk-wise from HBM and
contract against the (tiny, resident) X on the TensorEngine, producing
feats^T directly in PSUM; the MLP runs on-chip afterwards.  The kernel
is HBM-bandwidth bound: 64 MB per core.

Matmuls use float32r (fp32 bit layout, reduced-precision multiply at
1 cycle/row for moving dim >= 256) so the f32 HBM data feeds the PE
with zero conversion ops.
"""

import numpy as np

import concourse.bass as bass
import concourse.mybir as mybir
import concourse.tile as tile
from concourse.bass_utils import run_bass_kernel_spmd

B, N, E, F, H = 2, 4096, 8192, 16, 64
NCORES = 8
SPB = NCORES // B          # edge shards per batch = 4
EPC = E // SPB             # edges per core = 2048
P = 128                    # partition / node-chunk size
NCH = N // P               # 32 node chunks
CQ = 3                     # node chunks per big DMA transfer (3 MB)
# DMA groups: 3 MB transfers for the bulk, then two 1 MB tail transfers
# in their own (small) buffer slots so the final DMAs never wait on a
# big-group slot release and the PE tail after the last DMA is short.
GROUPS = [(g * CQ, CQ) for g in range(10)] + [(30, 1), (31, 1)]
NB = 512                   # PSUM bank width in f32
NCB = EPC // NB            # 4 column blocks per core

_f32 = mybir.dt.float32
_f32r = mybir.dt.float32r
_AF = mybir.ActivationFunctionType


def _build_nc() -> bass.Bass:
    nc = bass.Bass()

    x = nc.dram_tensor("x", [P, NCH * F], _f32r, kind="ExternalInput")
    ro = nc.dram_tensor("ro", [N, EPC], _f32r, kind="ExternalInput")
    ri = nc.dram_tensor("ri", [N, EPC], _f32r, kind="ExternalInput")
    w1 = nc.dram_tensor("w1", [2 * F, H], _f32r, kind="ExternalInput")
    b1 = nc.dram_tensor("b1", [H, 1], _f32, kind="ExternalInput")
    w2 = nc.dram_tensor("w2", [H, 1], _f32r, kind="ExternalInput")
    b2 = nc.dram_tensor("b2", [1, 1], _f32, kind="ExternalInput")
    out = nc.dram_tensor("out", [1, EPC], _f32, kind="ExternalOutput")

    # node dim 4096 -> (j, p): chunk j covers nodes 128j..128j+127;
    # partition p = node within chunk.
    ro_r = ro.rearrange("(j p) e -> j p e", p=P)
    ri_r = ri.rearrange("(j p) e -> j p e", p=P)

    with tile.TileContext(nc) as tc:
        with (
            tc.tile_pool(name="consts", bufs=1) as consts,
            tc.tile_pool(name="ro_pool", bufs=2) as ro_pool,
            tc.tile_pool(name="ri_pool", bufs=2) as ri_pool,
            tc.tile_pool(name="mid", bufs=1) as mid,
            tc.tile_pool(name="ps", bufs=8, space="PSUM") as ps,
        ):
            x_sb = consts.tile([P, NCH * F], _f32r)
            nc.gpsimd.dma_start(out=x_sb, in_=x.ap())
            b1_sb = consts.tile([H, 1], _f32)
            nc.gpsimd.dma_start(out=b1_sb, in_=b1.ap())
            w2_sb = consts.tile([H, 1], _f32r)
            nc.gpsimd.dma_start(out=w2_sb, in_=w2.ap())
            b2_sb = consts.tile([P, 1], _f32)
            nc.gpsimd.dma_start(out=b2_sb, in_=b2.ap().to_broadcast([P, 1]))

            # feats^T accumulators: (bo, bi) x 4 col-blocks = 8 PSUM banks.
            acc = {}
            for m in range(2):
                for cb in range(NCB):
                    acc[(m, cb)] = ps.tile([F, NB], _f32, tag="bank", name=f"acc_{m}_{cb}")

            for j0, cq in GROUPS:
                tail = "_tail" if cq < CQ else ""
                # [P, cq, EPC] view of chunks j0..j0+cq of each matrix
                ro_t = ro_pool.tile([P, cq, EPC], _f32r, tag="ro" + tail, name=f"ro_t_{j0}")
                nc.sync.dma_start(
                    out=ro_t,
                    in_=ro_r[j0 : j0 + cq].rearrange("c p e -> p c e"),
                )
                ri_t = ri_pool.tile([P, cq, EPC], _f32r, tag="ri" + tail, name=f"ri_t_{j0}")
                nc.scalar.dma_start(
                    out=ri_t,
                    in_=ri_r[j0 : j0 + cq].rearrange("c p e -> p c e"),
                )
                for c in range(cq):
                    j = j0 + c
                    lhsT = x_sb[:, bass.ts(j, F)]  # [128, 16]
                    first = j == 0
                    last = j == NCH - 1
                    for cb in range(NCB):
                        nc.tensor.matmul(
                            acc[(0, cb)],
                            lhsT,
                            ro_t[:, c, bass.ts(cb, NB)],
                            start=first,
                            stop=last,
                        )
                        nc.tensor.matmul(
                            acc[(1, cb)],
                            lhsT,
                            ri_t[:, c, bass.ts(cb, NB)],
                            start=first,
                            stop=last,
                        )

            # feats^T in two base-0 tiles (engines can only write at
            # 32-aligned base partitions): bo^T [F, EPC], bi^T [F, EPC].
            bo_sb = mid.tile([F, EPC], _f32r)
            bi_sb = mid.tile([F, EPC], _f32r)
            for cb in range(NCB):
                nc.vector.tensor_copy(
                    bo_sb[:, bass.ts(cb, NB)], acc[(0, cb)][:]
                )
                nc.vector.tensor_copy(
                    bi_sb[:, bass.ts(cb, NB)], acc[(1, cb)][:]
                )

            # W1 split into the bo half and the bi half, both at base
            # partition 0, so MLP layer 1 is two K=16 accumulating matmuls.
            w1a_sb = consts.tile([F, H], _f32r)
            nc.gpsimd.dma_start(out=w1a_sb, in_=w1.ap()[0:F, :])
            w1b_sb = consts.tile([F, H], _f32r)
            nc.gpsimd.dma_start(out=w1b_sb, in_=w1.ap()[F : 2 * F, :])

            h_sb = mid.tile([H, EPC], _f32r)
            out_sb = mid.tile([1, EPC], _f32)
            # PE is in-order: issue all layer-1 matmuls before any layer-2
            # matmul so PE never stalls waiting for a tanh of the same block.
            hps = []
            for cb in range(NCB):
                hp = ps.tile([H, NB], _f32, tag="bank", name=f"hp_{cb}")
                nc.tensor.matmul(
                    hp[:],
                    w1a_sb[:],
                    bo_sb[:, bass.ts(cb, NB)],
                    start=True,
                    stop=False,
                )
                nc.tensor.matmul(
                    hp[:],
                    w1b_sb[:],
                    bi_sb[:, bass.ts(cb, NB)],
                    start=False,
                    stop=True,
                )
                hps.append(hp)
            for cb in range(NCB):
                nc.scalar.activation(
                    h_sb[:, bass.ts(cb, NB)], hps[cb][:], _AF.Tanh, bias=b1_sb[:]
                )
            lps = []
            for cb in range(NCB):
                lp = ps.tile([1, NB], _f32, tag="bank", name=f"lp_{cb}")
                nc.tensor.matmul(
                    lp[:],
                    w2_sb[:],
                    h_sb[:, bass.ts(cb, NB)],
                    start=True,
                    stop=True,
                )
                lps.append(lp)
            for cb in range(NCB):
                nc.scalar.activation(
                    out_sb[:, bass.ts(cb, NB)],
                    lps[cb][:],
                    _AF.Sigmoid,
                    bias=b2_sb[0:1, :],
                )
            nc.sync.dma_start(out=out.ap(), in_=out_sb)

    return nc


def _split_multi_waits(nc: bass.Bass, limit: int = 1) -> None:
    """The walrus build in this image lowers at most one sync-wait per
    instruction ("Too many sync wait commands").  Move surplus waits onto
    standalone event-semaphore instructions inserted just before the
    over-subscribed instruction on the same engine — identical sync
    semantics, one wait per instruction."""
    n = 0
    for f in nc.m.functions:
        for bb in f.blocks:
            insts = bb.instructions  # live list
            new_list = []
            for inst in list(insts):
                si = inst.sync_info
                if si is not None and len(si.on_wait) > limit:
                    waits = list(si.on_wait)
                    extra, keep = waits[:-limit], waits[-limit:]
                    for w in extra:
                        n += 1
                        ev = mybir.InstEventSemaphore(
                            name=f"splitwait_{n}", ins=[], outs=[],
                            engine=inst.engine,
                        )
                        ev.sync_info = mybir.SyncInfo(on_wait=[w], on_update=[])
                        nc.register_instruction(ev, overwrite=True)
                        new_list.append(ev)
                    si.on_wait = keep
                new_list.append(inst)
            insts[:] = new_list


def _hoist_first_dmas(nc: bass.Bass) -> None:
    """Move the first Ro/Ri bulk DMAs (no sync waits; consumers gate on
    their completion semaphores) from the kernel block into the prologue
    block, ahead of the ~4 us engine-boot barrier, so the HBM stream
    starts as soon as the SP/ACT sequencers are up."""
    blocks = nc.m.functions[0].blocks
    if len(blocks) < 2:
        return
    bb0, bb1 = blocks[0], blocks[1]
    hoisted = []
    seen_engines = set()
    b1_insts = bb1.instructions
    keep = []
    for inst in list(b1_insts):
        eng = str(inst.engine)
        if (
            type(inst).__name__ == "InstDMACopy"
            and eng in ("EngineType.SP", "EngineType.Activation")
            and eng not in seen_engines
            and (inst.sync_info is None or not inst.sync_info.on_wait)
        ):
            seen_engines.add(eng)
            hoisted.append(inst)
        else:
            keep.append(inst)
    if not hoisted:
        return
    b1_insts[:] = keep
    b0 = bb0.instructions
    # insert before the first Drain (the boot barrier) in block 0
    pos = next(
        (i for i, inst in enumerate(b0) if type(inst).__name__ == "InstDrain"),
        len(b0),
    )
    b0[:] = b0[:pos] + hoisted + b0[pos:]


_NC_CACHE = None


def _get_nc() -> bass.Bass:
    global _NC_CACHE
    if _NC_CACHE is None:
        nc = _build_nc()
        _hoist_first_dmas(nc)
        _split_multi_waits(nc)
        _NC_CACHE = nc
    return _NC_CACHE


def _make_in_maps(X, Ri, Ro, W1, b1, W2, b2):
    X = np.asarray(X, np.float32)
    W1 = np.asarray(W1, np.float32)
    b1 = np.asarray(b1, np.float32)
    W2 = np.asarray(W2, np.float32)
    b2 = np.asarray(b2, np.float32)
    in_maps = []
    for core in range(NCORES):
        b = core // SPB
        e0 = (core % SPB) * EPC
        # pack X[b] so partition p, cols [16j:16j+16] = X[b, 128j + p, :]
        xp = np.ascontiguousarray(
            X[b].reshape(NCH, P, F).transpose(1, 0, 2).reshape(P, NCH * F)
        )
        in_maps.append(
            {
                "x": xp,
                "ro": np.ascontiguousarray(np.asarray(Ro)[b, :, e0 : e0 + EPC]),
                "ri": np.ascontiguousarray(np.asarray(Ri)[b, :, e0 : e0 + EPC]),
                "w1": W1,
                "b1": b1.reshape(H, 1),
                "w2": W2.reshape(H, 1),
                "b2": b2.reshape(1, 1),
            }
        )
    return in_maps


def run(inputs: dict, trace: bool = False, trace_cores=None):
    """Run the kernel; returns (full_output, BassKernelResults)."""
    nc = _get_nc()
    in_maps = _make_in_maps(**inputs)
    bkr = run_bass_kernel_spmd(
        nc,
        in_maps,
        core_ids=list(range(NCORES)),
        trace=trace,
        trace_cores=trace_cores,
    )
    out = np.empty((B, E), np.float32)
    for core in range(NCORES):
        b = core // SPB
        e0 = (core % SPB) * EPC
        out[b, e0 : e0 + EPC] = bkr.results[core]["out"].reshape(EPC)
    return out, bkr


def kernel(**inputs) -> np.ndarray:
    out, _ = run(inputs)
    return out
